# revision 1
# baseline (speedup 1.0000x reference)
"""Trainium2 Bass kernel for nn_DentalAnatomyLoss.

Computes, for segmentation [B=2, C=32, D=64, H=128, W=128] fp32:
  - crown/root ratio loss (per (b,c) sums over d<32 / d>=32)
  - 3D total-variation loss (mean |diff| along w, h, d)
  - returns stack([crown_root, smoothness, total_anatomy]) fp32 [3]

Strategy: pure data-parallel over the 64 (b,c) slices, 8 per NeuronCore.
Each core reduces its 32 MiB shard to a [128, 160] fp32 partial tensor;
the host combines partials into the 3 scalars.

Per-core engine split (memory regime, ~94 us HBM roofline/core):
  - ScalarE: fp32->bf16 cast with fused accum_out (crown/root sums), and
    Abs+accum_out consuming the h-diff matmul output from PSUM.
  - VectorE: the w-diff as one fused scalar_tensor_tensor (out=max(a,b),
    accum_out=sum) reading fp32 directly (the shift-by-one AP is 1x in
    any dtype); the d-diff as an aligned 2x subtract + 4x fused relu-sum.
    The host recovers sum|a-b| = 2*sum(max(a,b)) - sum(a) - sum(b) and
    sum|d| = 2*sum(max(d,0)) - sum(d), with the signed sums telescoping
    to boundary-column sums.
  - TensorE: bidiagonal matmul computes h-diffs (partition axis) in PSUM.
  - DMA: HBM loads only (the SP ring), ~94 us/core at ~360 GB/s.

Pipelining: xb-dependent work (d-diff, h-diff matmul) is emitted one
chunk late so VectorE never waits on the cast; PSUM is two half-chunk
tiles (4 banks each) so TensorE fills one while ScalarE drains the
other; each PSUM drain is deferred past the next fill.
"""

import os

import numpy as np

B, C, D, H, W = 2, 32, 64, 128, 128
NCORES = 8
JPC = (B * C) // NCORES  # (b,c) slices per core
CROWN_ROOT_W = 2.0
SMOOTH_W = 1.5
EXPECTED_RATIO = 1.2

# accumulator column layout in the [128, ACC_COLS] partial tensor
# (one column per chunk = (slice j, half); 16 chunks per core)
ACC_COLS = 160
COL_X = 0  # 16: sum(x) per chunk
COL_DXP = 16  # 16: sum(max(x[...,w], x[...,w+1])) over w-pairs
COL_TXF = 32  # 16: sum over planes of column w=0
COL_TXL = 48  # 16: sum over planes of column w=W-1
COL_DZP = 64  # 16: sum(max(dz,0)), dz = plane[k+1]-plane[k] (in-chunk)
COL_TZF = 80  # 16: sum of first plane of chunk
COL_TZL = 96  # 16: sum of last plane of chunk
COL_DY = 112  # 32: sum|dy| per (chunk, psum-half)
COL_BNDP = 144  # 8: sum(max(a,b)) for the half0/half1 boundary plane pair
# 152:160 unused (zeroed)

_PROG_CACHE: dict = {}
last_exec_time_ns = None  # set by kernel() when tracing is enabled


def _build_program(jpc=JPC, d=D, h=H, w=W, repeat=1, skip=()):
    """Build the (single) SPMD Bass program run identically on all cores.

    repeat>1 wraps the whole compute in a hardware For_i loop (identical
    result, used only for wall-clock timing of the kernel body).
    """
    from contextlib import ExitStack

    import concourse.tile as tile
    from concourse import bacc, mybir

    f32 = mybir.dt.float32
    bf16 = mybir.dt.bfloat16
    AO = mybir.AluOpType
    AF = mybir.ActivationFunctionType

    ndh = d // 2  # planes per chunk; chunks never straddle the crown/root split
    fsz = ndh * w  # free size of one chunk

    nc = bacc.Bacc(
        "TRN2",
        target_bir_lowering=False,
        debug=False,
        enable_asserts=False,
        num_devices=NCORES,
    )
    seg = nc.dram_tensor("seg", [jpc, d, h, w], f32, kind="ExternalInput").ap()
    bd = nc.dram_tensor("bidiag", [h, h], bf16, kind="ExternalInput").ap()
    out = nc.dram_tensor("partials", [h, ACC_COLS], f32, kind="ExternalOutput").ap()

    with tile.TileContext(nc) as tc, ExitStack() as ctx:
        singles = ctx.enter_context(tc.tile_pool(name="singles", bufs=1))
        x32p = ctx.enter_context(tc.tile_pool(name="x32", bufs=3))
        xbp = ctx.enter_context(tc.tile_pool(name="xb", bufs=4))
        dxp = ctx.enter_context(tc.tile_pool(name="dx", bufs=2))
        dzp = ctx.enter_context(tc.tile_pool(name="dz", bufs=2))
        tinyp = ctx.enter_context(tc.tile_pool(name="tiny", bufs=2))
        dummyp = ctx.enter_context(tc.tile_pool(name="dummy", bufs=4))
        psp = ctx.enter_context(tc.tile_pool(name="ps", bufs=2, space="PSUM"))

        bd_sb = singles.tile([h, h], bf16)
        nc.sync.dma_start(out=bd_sb, in_=bd)
        acc = singles.tile([h, ACC_COLS], f32)
        nc.vector.memset(acc, 0.0)

        nblk = fsz // 512  # matmul free-dim blocks (512 = one PSUM bank)
        planes_per_blk = 512 // w
        nsub = 2 if nblk % 2 == 0 and nblk >= 2 else 1
        hb = nblk // nsub  # psum blocks per half-chunk tile

        def sum_max(out_ap, a_ap, b_ap, col):
            """out = max(a,b); acc[:,col] = sum(out). out is write-only."""
            nc.vector.scalar_tensor_tensor(
                out=out_ap,
                in0=a_ap,
                scalar=0.0,
                in1=b_ap,
                op0=AO.bypass,
                op1=AO.max,
                accum_out=acc[:, col : col + 1],
            )

        def sum_relu(src_ap, col):
            """acc[:,col] = sum(max(src,0)); src rewritten in place."""
            nc.vector.tensor_scalar(
                out=src_ap,
                in0=src_ap,
                scalar1=0.0,
                scalar2=None,
                op0=AO.max,
                op1=AO.add,
                accum_out=acc[:, col : col + 1],
            )

        def sum_ident(src_ap, col):
            """acc[:,col] = sum(src); src rewritten in place (x + 0.0).

            Only used on tiles of non-negative values (x in [0,1)), so the
            identity rewrite is bit-exact.
            """
            nc.vector.tensor_scalar(
                out=src_ap,
                in0=src_ap,
                scalar1=0.0,
                scalar2=None,
                op0=AO.add,
                op1=AO.add,
                accum_out=acc[:, col : col + 1],
            )

        state = {"prev_xb": None, "pending_gy": None, "pending_c": None}

        def emit_gy(ps_tile, cidx, sub):
            dya = dummyp.tile([h, 1], bf16)
            col = COL_DY + nsub * cidx + sub
            nc.scalar.activation(
                out=dya.broadcast_to((h, hb, 512)),
                in_=ps_tile[:, :, :],
                func=AF.Abs,
                accum_out=acc[:, col : col + 1],
            )

        def stage_c(j, half, cidx, xb, xbf):
            """xb-dependent work, emitted one chunk late (see module doc)."""
            # h-diff (gy) via bidiagonal matmul into PSUM; two half-chunk
            # tiles so PE fills one while ScalarE drains the other, and each
            # drain is deferred past the next fill.
            if "gy" not in skip:
                for sub in range(nsub):
                    ps = psp.tile([h, hb, 512], f32)
                    for blk in range(hb):
                        g = sub * hb + blk
                        nc.tensor.matmul(
                            ps[:, blk, :],
                            bd_sb,
                            xb[:, g * planes_per_blk : (g + 1) * planes_per_blk, :],
                            start=True,
                            stop=True,
                        )
                    if state["pending_gy"] is not None:
                        emit_gy(*state["pending_gy"])
                    state["pending_gy"] = (ps, cidx, sub)

            # d-diff (gz), in-chunk pairs: aligned TT subtract (2x) then
            # fused relu-sum (4x); sum(dz) telescopes on host.
            if "dz" not in skip:
                dz = dzp.tile([h, fsz - w], bf16)
                nc.vector.tensor_tensor(
                    out=dz,
                    in0=xbf[:, w:fsz],
                    in1=xbf[:, 0 : fsz - w],
                    op=AO.subtract,
                )
                sum_relu(dz[:, :], COL_DZP + cidx)
                # first/last plane sums for the signed sums
                sum_ident(xb[:, 0, :], COL_TZF + cidx)
                sum_ident(xb[:, ndh - 1, :], COL_TZL + cidx)

                # boundary pair between the two halves of slice j
                if half == 1:
                    bnd = tinyp.tile([h, w], bf16)
                    sum_max(
                        bnd,
                        xb[:, 0, :],
                        state["prev_xb"][:, ndh - 1, :],
                        COL_BNDP + j,
                    )
                state["prev_xb"] = xb

        def chunk_body(j, half):
            cidx = j * 2 + half
            d0 = half * ndh

            # 1) load chunk: [h partitions, ndh planes, w] fp32
            x32 = x32p.tile([h, ndh, w], f32)
            nc.sync.dma_start(
                out=x32, in_=seg[j, d0 : d0 + ndh, :, :].rearrange("d h w -> h d w")
            )

            # 2) cast to bf16; fused accum -> crown/root sum for this chunk
            if "conv" in skip:
                return
            xb = xbp.tile([h, ndh, w], bf16)
            nc.scalar.activation(
                out=xb,
                in_=x32,
                func=AF.Copy,
                accum_out=acc[:, COL_X + cidx : COL_X + cidx + 1],
            )
            xbf = xb.rearrange("p a b -> p (a b)")

            # 3) w-diff (gx): one fused op per chunk.  The exact 3D AP
            #    (misaligned by one element) runs at 1x either way, so it
            #    reads the fp32 tile directly: no dependency on the cast,
            #    and full fp32 precision for the gx term.
            # 4) run the previous chunk's deferred xb-dependent work FIRST:
            #    it is ready now, while this chunk's dx still waits on its
            #    DMA -- this order lets VectorE cover DMA latency
            if state["pending_c"] is not None:
                stage_c(*state["pending_c"])
            state["pending_c"] = (j, half, cidx, xb, xbf)

            if "dx" not in skip:
                dx = dxp.tile([h, ndh, w - 1], bf16)
                sum_max(dx, x32[:, :, 1:], x32[:, :, 0 : w - 1], COL_DXP + cidx)
                # boundary-column sums for the signed sums (fp32)
                sum_ident(x32[:, :, 0:1], COL_TXF + cidx)
                sum_ident(x32[:, :, w - 1 : w], COL_TXL + cidx)

        def all_chunks():
            for j in range(jpc):
                for half in range(2):
                    chunk_body(j, half)
            if state["pending_c"] is not None:
                stage_c(*state["pending_c"])
            state["pending_c"] = None
            if state["pending_gy"] is not None:
                emit_gy(*state["pending_gy"])
            state["pending_gy"] = None

        if repeat == 1:
            all_chunks()
        else:
            with tc.For_i(0, repeat, 1):
                all_chunks()
        nc.sync.dma_start(out=out, in_=acc)

    nc.compile()
    return nc


def _get_program():
    key = "full"
    if key not in _PROG_CACHE:
        _PROG_CACHE[key] = _build_program()
    return _PROG_CACHE[key]


def _bidiag_np(h=H):
    """lhsT for the h-diff matmul: out[m,:] = rhs[m+1,:] - rhs[m,:]."""
    import ml_dtypes

    m = np.zeros((h, h), dtype=np.float32)
    for c in range(h - 1):
        m[c + 1, c] = 1.0
        m[c, c] = -1.0
    # last column stays zero -> output row h-1 is 0
    return m.astype(ml_dtypes.bfloat16)


def _combine(partials, b=B, c=C, d=D, h=H, w=W):
    """Host-side finish: per-core [128, 160] fp32 partials -> [3] fp32."""
    nslice = b * c
    jpc = nslice // len(partials)

    crown = np.zeros(nslice, dtype=np.float64)
    root = np.zeros(nslice, dtype=np.float64)
    gx_sum = 0.0
    gy_sum = 0.0
    gz_sum = 0.0
    for k, p in enumerate(partials):
        p = p.astype(np.float64)
        xp = p[:, COL_DXP : COL_DXP + 2 * jpc].sum(axis=0)
        txf = p[:, COL_TXF : COL_TXF + 2 * jpc].sum(axis=0)
        txl = p[:, COL_TXL : COL_TXL + 2 * jpc].sum(axis=0)
        zp = p[:, COL_DZP : COL_DZP + 2 * jpc].sum(axis=0)
        tzf = p[:, COL_TZF : COL_TZF + 2 * jpc].sum(axis=0)
        tzl = p[:, COL_TZL : COL_TZL + 2 * jpc].sum(axis=0)
        bndp = p[:, COL_BNDP : COL_BNDP + jpc].sum(axis=0)

        xs = p[:, COL_X : COL_X + 2 * jpc].sum(axis=0)
        # sum|a-b| = 2*sum(max(a,b)) - sum(a) - sum(b)
        # gx: a = x[..., 1:], b = x[..., :-1]
        gx_sum += (2.0 * xp - (xs - txf) - (xs - txl)).sum()
        # gz: dz = planes[1:] - planes[:-1]; sum(dz) = tzl - tzf
        gz_sum += (2.0 * zp - (tzl - tzf)).sum()
        # boundary pair: a = half1.plane0, b = half0.plane(ndh-1)
        for jj in range(jpc):
            gz_sum += 2.0 * bndp[jj] - tzf[2 * jj + 1] - tzl[2 * jj]
        gy_sum += p[:, COL_DY : COL_DY + 4 * jpc].sum()

        for jj in range(jpc):
            crown[k * jpc + jj] = p[:, COL_X + 2 * jj].sum()
            root[k * jpc + jj] = p[:, COL_X + 2 * jj + 1].sum()

    total = crown + root
    valid = (total > 0) & (root > 0)
    safe_root = np.where(root > 0, root, 1.0)
    ratio_loss = np.where(valid, (crown / safe_root - EXPECTED_RATIO) ** 2, 0.0)
    cr_loss = ratio_loss.sum() / nslice

    nx = nslice * d * h * (w - 1)
    ny = nslice * d * (h - 1) * w
    nz = nslice * (d - 1) * h * w
    tv = gx_sum / nx + gy_sum / ny + gz_sum / nz

    crown_root = cr_loss * CROWN_ROOT_W
    smoothness = tv * SMOOTH_W
    return np.array(
        [crown_root, smoothness, crown_root + smoothness], dtype=np.float32
    )


def kernel(segmentation: np.ndarray) -> np.ndarray:
    global last_exec_time_ns
    from concourse.bass_utils import run_bass_kernel_spmd

    seg = np.ascontiguousarray(np.asarray(segmentation), dtype=np.float32)
    assert seg.shape == (B, C, D, H, W)
    nc = _get_program()

    bd = _bidiag_np()
    shards = seg.reshape(B * C, D, H, W)
    in_maps = [
        {"seg": np.ascontiguousarray(shards[k * JPC : (k + 1) * JPC]), "bidiag": bd}
        for k in range(NCORES)
    ]
    trace = bool(os.environ.get("BASS_TRACE"))
    res = run_bass_kernel_spmd(nc, in_maps, list(range(NCORES)), trace=trace)
    last_exec_time_ns = res.exec_time_ns
    partials = [res.results[k]["partials"] for k in range(NCORES)]
    return _combine(partials)



# revision 12
# speedup vs baseline: 1.8151x; 1.8151x over previous
"""Trainium2 Bass kernel for nn_DentalAnatomyLoss.

For segmentation [B=2, C=32, D=64, H=128, W=128] fp32 computes
  - crown/root ratio loss (per (b,c) sums over d<32 / d>=32)
  - 3D total-variation loss (mean |diff| along w, h, d)
  - returns stack([crown_root, smoothness, total_anatomy]) fp32 [3]

Data-parallel over the 64 (b,c) slices, 8 per NeuronCore.  Per core the
shard is laid out so that every engine works near its throughput cap:

  partition p = jq*32 + dhi   (jq: 4 slice-pairs, dhi: 32 d-plane pairs)
  free axis   = (dlo 2, h 128, w 128, jlo 2)   [jlo: 2 slices interleaved]

The jlo-innermost interleave makes the w-diff a shift-by-2-element
(4-byte aligned) bf16 tensor_tensor -> 2x DVE mode, likewise the h-diff
(shift 256) and the even d-pairs (shift 32768).  Odd d-pairs cross
partitions and go through one masked shift-matmul pair on TensorE with
an Abs+accum PSUM drain on ScalarE.  sum|a-b| is recovered host-side
via 2*sum(max(a,b)) - sum(a) - sum(b); the big max-scratch sums ride
TensorE (ones-matmul accumulated into persistent PSUM chains), so DVE
only pays 0.5 cyc/elem per diff.

Engine budget per core (32 MiB shard, ~94 us HBM roofline):
  DMA 94us (16 KiB contiguous descriptors) | ScalarE ~91us (cast +
  drains) | DVE ~90us (three 2x TT passes) | PE ~88us (shift-matmuls +
  ones-sums).
"""

import os

import numpy as np

B, C, D, H, W = 2, 32, 64, 128, 128
NCORES = 8
JPC = (B * C) // NCORES  # slices per core = 8
CROWN_ROOT_W = 2.0
SMOOTH_W = 1.5
EXPECTED_RATIO = 1.2

NJQ, NJLO = 4, 2  # slice split: j = jq*2 + jlo
NDHI, NDLO = 32, 2  # plane split: d = 2*dhi + dlo
NHQ = 4  # h stream chunks
HS = H // NHQ  # rows per chunk

# accumulator column layout ([128, ACC_COLS] fp32 partials per core)
ACC_COLS = 96
COL_CAST = 0  # 16: sum(xb) per (hq, dlo, jlo) chunk
COL_GXW0 = 16  # 4: sum over (dlo,hs,jlo) of w=0 col, per hq
COL_GXW1 = 20  # 4: same for w=127
COL_GYH0 = 24  # 1: sum of h=0 row (all dlo,jlo)
COL_GYH1 = 25  # 1: sum of h=127 row
COL_GZE = 26  # 4: sum(max(dlo1, dlo0)) per hq
COL_CHX = 30  # 1: global sum(max) for gx (PE chain, all rows equal)
COL_CHY = 31  # 1: global sum(max) for gy + h-boundary pairs
COL_GZO = 32  # 32: |odd d-pair diff| sums per (hq, 1024-group)
# 64:96 unused

_PROG_CACHE: dict = {}
last_exec_time_ns = None


def _build_program(repeat=1, hq_chunks=NHQ, skip=()):
    from contextlib import ExitStack

    import concourse.tile as tile
    from concourse import bacc, mybir

    f32 = mybir.dt.float32
    bf16 = mybir.dt.bfloat16
    AO = mybir.AluOpType
    AF = mybir.ActivationFunctionType

    nhq = hq_chunks
    hs = H // nhq
    grp = hs * W * NJLO  # free els per (dlo) per chunk-group = 8192 @ hs=32

    nc = bacc.Bacc(
        "TRN2",
        target_bir_lowering=False,
        debug=False,
        enable_asserts=False,
        num_devices=NCORES,
    )
    # host pre-transposed shard: [jlo, dlo, hq, p=jq*32+dhi, hs, w], so each
    # chunk load is one 128-partition DMA with 16 KiB contiguous runs
    seg = nc.dram_tensor(
        "seg", [NJLO, NDLO, nhq, 128, hs, W], f32, kind="ExternalInput"
    ).ap()
    mats = nc.dram_tensor("mats", [128, 3, 128], bf16, kind="ExternalInput").ap()
    out = nc.dram_tensor("partials", [128, ACC_COLS], f32, kind="ExternalOutput").ap()

    with tile.TileContext(nc) as tc, ExitStack() as ctx:
        singles = ctx.enter_context(tc.tile_pool(name="singles", bufs=1))
        x32p = ctx.enter_context(tc.tile_pool(name="x32", bufs=3))
        xbgp = ctx.enter_context(tc.tile_pool(name="xbg", bufs=2))
        scrp = ctx.enter_context(tc.tile_pool(name="scr", bufs=4))
        tinyp = ctx.enter_context(tc.tile_pool(name="tiny", bufs=3))
        dummyp = ctx.enter_context(tc.tile_pool(name="dummy", bufs=2))
        psp = ctx.enter_context(tc.tile_pool(name="ps", bufs=2, space="PSUM"))
        pschain = ctx.enter_context(tc.tile_pool(name="chain", bufs=1, space="PSUM"))

        mats_sb = singles.tile([128, 3, 128], bf16)
        nc.sync.dma_start(out=mats_sb, in_=mats)
        lhs_shift = mats_sb[:, 0, :]
        lhs_negi = mats_sb[:, 1, :]
        lhs_ones = mats_sb[:, 2, :]
        acc = singles.tile([128, ACC_COLS], f32)
        nc.vector.memset(acc, 0.0)
        zs = singles.tile([128, 512], bf16)
        nc.vector.memset(zs, 0.0)

        chx = pschain.tile([128, 1024], f32)
        chy = pschain.tile([128, 1024], f32)
        state = {"chx_n": 0, "chy_n": 0, "prev_xbg": None}

        def chain_open(ps):
            for half in range(2):
                nc.tensor.matmul(
                    ps[:, half * 512 : (half + 1) * 512],
                    lhs_ones,
                    zs,
                    start=True,
                    stop=False,
                    skip_group_check=True,
                )

        def chain_mm(ps, key, flat_ap, n):
            """Accumulate ones-matmul column-sums of flat_ap into ps.

            Each matmul's out must stay inside one PSUM bank (512 fp32);
            alternate bank-halves of ps, with per-half start tracking.
            """
            g0 = 0
            while g0 < n:
                g = min(512, n - g0)
                half = state[key + "_n"] % 2
                nc.tensor.matmul(
                    ps[:, half * 512 : half * 512 + g],
                    lhs_ones,
                    flat_ap[:, g0 : g0 + g],
                    start=False,
                    stop=False,
                    skip_group_check=True,
                )
                state[key + "_n"] += 1
                g0 += g

        def chain_close(ps, key, col):
            # zero-rhs matmul with stop to close the accumulation group
            nc.tensor.matmul(
                ps[:, 0:512],
                lhs_ones,
                zs,
                start=False,
                stop=True,
                skip_group_check=True,
            )
            dy = dummyp.tile([128, 1], bf16)
            nc.scalar.activation(
                out=dy.broadcast_to((128, 1024)),
                in_=ps,
                func=AF.Copy,
                accum_out=acc[:, col : col + 1],
            )

        def sum_small(in_ap, col):
            """acc[:,col] = sum(in_ap) via tensor_scalar into a tiny out."""
            t = tinyp.tile([128] + list(in_ap.shape[1:]), bf16)
            nc.vector.tensor_scalar(
                out=t,
                in0=in_ap,
                scalar1=0.0,
                scalar2=None,
                op0=AO.add,
                op1=AO.add,
                accum_out=acc[:, col : col + 1],
            )

        def group(hq):
            xbg = xbgp.tile([128, NDLO, hs, W, NJLO], bf16)
            for dlo in range(NDLO):
                for jlo in range(NJLO):
                    x32 = x32p.tile([128, hs, W], f32)
                    nc.sync.dma_start(out=x32, in_=seg[jlo, dlo, hq])
                    if "conv" in skip:
                        continue
                    nc.scalar.activation(
                        out=xbg[:, dlo, :, :, jlo],
                        in_=x32,
                        func=AF.Copy,
                        accum_out=acc[
                            :,
                            COL_CAST + (hq * NDLO + dlo) * NJLO + jlo
                            : COL_CAST + (hq * NDLO + dlo) * NJLO + jlo + 1,
                        ],
                    )
            if "conv" in skip:
                return

            for dlo in range(NDLO):
                if "gx" not in skip:
                    # w-diff pairs: shift by jlo width (4B) -> 2x TT
                    scr = scrp.tile([128, hs, W - 1, NJLO], bf16)
                    nc.vector.tensor_tensor(
                        out=scr,
                        in0=xbg[:, dlo, :, 1:, :],
                        in1=xbg[:, dlo, :, : W - 1, :],
                        op=AO.max,
                    )
                    chain_mm(chx, "chx",
                             scr.rearrange("p a b c -> p (a b c)"),
                             hs * (W - 1) * NJLO)
                if "gy" not in skip:
                    # h-diff pairs inside this chunk
                    scr = scrp.tile([128, hs - 1, W, NJLO], bf16)
                    nc.vector.tensor_tensor(
                        out=scr,
                        in0=xbg[:, dlo, 1:, :, :],
                        in1=xbg[:, dlo, : hs - 1, :, :],
                        op=AO.max,
                    )
                    chain_mm(chy, "chy",
                             scr.rearrange("p a b c -> p (a b c)"),
                             (hs - 1) * W * NJLO)
                    # boundary row-pair with the previous chunk
                    if hq > 0:
                        scrb = tinyp.tile([128, 1, W, NJLO], bf16)
                        nc.vector.tensor_tensor(
                            out=scrb,
                            in0=xbg[:, dlo, 0:1, :, :],
                            in1=state["prev_xbg"][:, dlo, hs - 1 : hs, :, :],
                            op=AO.max,
                        )
                        chain_mm(chy, "chy",
                                 scrb.rearrange("p a b c -> p (a b c)"),
                                 W * NJLO)

            if "gx" not in skip:
                sum_small(xbg[:, :, :, 0, :], COL_GXW0 + hq)
                sum_small(xbg[:, :, :, W - 1, :], COL_GXW1 + hq)
            if "gy" not in skip:
                if hq == 0:
                    sum_small(xbg[:, :, 0, :, :], COL_GYH0)
                if hq == nhq - 1:
                    sum_small(xbg[:, :, hs - 1, :, :], COL_GYH1)

            if "gze" not in skip:
                # even d-pairs: dlo1 vs dlo0, same partition, aligned
                scr = scrp.tile([128, hs, W, NJLO], bf16)
                nc.vector.tensor_tensor(
                    out=scr,
                    in0=xbg[:, 1, :, :, :],
                    in1=xbg[:, 0, :, :, :],
                    op=AO.max,
                )
                # fused 4x sum of the max-scratch on DVE
                nc.vector.tensor_scalar(
                    out=scr,
                    in0=scr,
                    scalar1=0.0,
                    scalar2=None,
                    op0=AO.add,
                    op1=AO.add,
                    accum_out=acc[:, COL_GZE + hq : COL_GZE + hq + 1],
                )

            if "gzo" not in skip:
                # odd d-pairs: x[p+1, dlo0] - x[p, dlo1] via masked
                # shift-matmul pair, Abs+accum drain from PSUM
                r0 = xbg[:, 0].rearrange("p a b c -> p (a b c)")
                r1 = xbg[:, 1].rearrange("p a b c -> p (a b c)")
                ngrp = grp // 1024
                for g in range(ngrp):
                    ps = psp.tile([128, 1024], f32)
                    for h2 in range(2):
                        c0 = g * 1024 + h2 * 512
                        nc.tensor.matmul(
                            ps[:, h2 * 512 : (h2 + 1) * 512],
                            lhs_shift, r0[:, c0 : c0 + 512],
                            start=True, stop=False,
                        )
                        nc.tensor.matmul(
                            ps[:, h2 * 512 : (h2 + 1) * 512],
                            lhs_negi, r1[:, c0 : c0 + 512],
                            start=False, stop=True,
                        )
                    dy = dummyp.tile([128, 1], bf16)
                    nc.scalar.activation(
                        out=dy.broadcast_to((128, 1024)),
                        in_=ps,
                        func=AF.Abs,
                        accum_out=acc[
                            :, COL_GZO + hq * ngrp + g : COL_GZO + hq * ngrp + g + 1
                        ],
                    )

            state["prev_xbg"] = xbg

        def body():
            state["chx_n"] = 0
            state["chy_n"] = 0
            state["prev_xbg"] = None
            if "gx" not in skip:
                chain_open(chx)
            if "gy" not in skip:
                chain_open(chy)
            for hq in range(nhq):
                group(hq)
            if "gx" not in skip:
                chain_close(chx, "chx", COL_CHX)
            if "gy" not in skip:
                chain_close(chy, "chy", COL_CHY)

        if repeat == 1:
            body()
        else:
            with tc.For_i(0, repeat, 1):
                body()
        nc.sync.dma_start(out=out, in_=acc)

    nc.compile()
    return nc


def _get_program():
    if "full" not in _PROG_CACHE:
        _PROG_CACHE["full"] = _build_program()
    return _PROG_CACHE["full"]


def _mats_np():
    """lhsT constants: [128, 3, 128] bf16 = [shift+1 masked, -I masked, ones]."""
    import ml_dtypes

    m = np.zeros((128, 3, 128), dtype=np.float32)
    for col in range(128):
        if col % 32 <= 30:  # valid odd pair: dhi <= 30
            m[col + 1, 0, col] = 1.0
            m[col, 1, col] = -1.0
    m[:, 2, :] = 1.0
    return m.astype(ml_dtypes.bfloat16)


def _combine(partials, nhq=NHQ):
    """Host-side finish: per-core [128, 96] fp32 partials -> [3] fp32."""
    ncores = len(partials)
    nslice = ncores * JPC

    crown = np.zeros(nslice, dtype=np.float64)
    root = np.zeros(nslice, dtype=np.float64)
    gx_sum = gy_sum = gz_sum = 0.0
    for k, p in enumerate(partials):
        p = p.astype(np.float64)
        cast = p[:, COL_CAST : COL_CAST + nhq * NDLO * NJLO]  # [128, 16]
        s_all = cast.sum()
        # per-slice crown/root: partition p=(jq,dhi), col c=(hq,dlo,jlo)
        for jq in range(NJQ):
            for jlo in range(NJLO):
                j = jq * NJLO + jlo
                rows = slice(jq * 32, (jq + 1) * 32)
                sub = cast[rows, jlo::NJLO]  # [32 dhi, nhq*NDLO]
                crown[k * JPC + j] = sub[:16].sum()
                root[k * JPC + j] = sub[16:].sum()
        # gx: 2*summax - (S - S_w0) - (S - S_w127)
        sw0 = p[:, COL_GXW0 : COL_GXW0 + nhq].sum()
        sw1 = p[:, COL_GXW1 : COL_GXW1 + nhq].sum()
        gx_sum += 2.0 * p[0, COL_CHX] - (s_all - sw0) - (s_all - sw1)
        sh0 = p[:, COL_GYH0].sum()
        sh1 = p[:, COL_GYH1].sum()
        gy_sum += 2.0 * p[0, COL_CHY] - (s_all - sh0) - (s_all - sh1)
        # gz even: 2*summax - S(dlo=1) - S(dlo=0)
        cast4 = cast.reshape(128, nhq, NDLO, NJLO)
        s_d0 = cast4[:, :, 0, :].sum()
        s_d1 = cast4[:, :, 1, :].sum()
        gze = 2.0 * p[:, COL_GZE : COL_GZE + nhq].sum() - s_d1 - s_d0
        gzo = p[:, COL_GZO : COL_GZO + 32].sum()
        gz_sum += gze + gzo

    total = crown + root
    valid = (total > 0) & (root > 0)
    safe_root = np.where(root > 0, root, 1.0)
    ratio_loss = np.where(valid, (crown / safe_root - EXPECTED_RATIO) ** 2, 0.0)
    cr_loss = ratio_loss.sum() / nslice

    nx = nslice * D * H * (W - 1)
    ny = nslice * D * (H - 1) * W
    nz = nslice * (D - 1) * H * W
    tv = gx_sum / nx + gy_sum / ny + gz_sum / nz

    crown_root = cr_loss * CROWN_ROOT_W
    smoothness = tv * SMOOTH_W
    return np.array(
        [crown_root, smoothness, crown_root + smoothness], dtype=np.float32
    )


def _prep_shard(shard, nhq=NHQ):
    """[8, D, H, W] -> [jlo, dlo, hq, p=jq*32+dhi, hs, w] contiguous."""
    hs = H // nhq
    a = shard.reshape(NJQ, NJLO, NDHI, NDLO, nhq, hs, W)
    a = a.transpose(1, 3, 4, 0, 2, 5, 6)  # jlo dlo hq jq dhi hs w
    return np.ascontiguousarray(a.reshape(NJLO, NDLO, nhq, 128, hs, W))


def kernel(segmentation: np.ndarray) -> np.ndarray:
    global last_exec_time_ns
    from concourse.bass_utils import run_bass_kernel_spmd

    seg = np.ascontiguousarray(np.asarray(segmentation), dtype=np.float32)
    assert seg.shape == (B, C, D, H, W)
    nc = _get_program()

    mats = _mats_np()
    shards = seg.reshape(B * C, D, H, W)
    in_maps = [
        {"seg": _prep_shard(shards[k * JPC : (k + 1) * JPC]), "mats": mats}
        for k in range(NCORES)
    ]
    trace = bool(os.environ.get("BASS_TRACE"))
    res = run_bass_kernel_spmd(nc, in_maps, list(range(NCORES)), trace=trace)
    last_exec_time_ns = res.exec_time_ns
    partials = [res.results[k]["partials"] for k in range(NCORES)]
    return _combine(partials)
